# revision 38
# baseline (speedup 1.0000x reference)
"""AttentionEncoder TRN2 Bass kernel (bf16 matmul path).

Data-parallel over batch: B=8 samples -> 8 NeuronCores, one sample per core.
All matmul operands are bf16 (host-cast, free); PSUM accumulation stays fp32.
res (LN1 output) stays resident in SBUF as bf16 -- no DRAM spill/reload.

Per-core pipeline (S=2048, D=1024, K=64):
  phase 0: kqT+biasT projections streamed against the xt DMA (chunk k of the
           contraction only needs xt[:,k,:]), then values s-major with
           stationary xT slices; scores(sb=0) interleaved into the values
           tail so the sigmoid pipeline fills for free.
  phase 1: per 512-col superblock: scoresT = sigmoid(qk+bias) via a 65-row
           contraction (row 64 = biasesT x ones), emitted one superblock
           ahead in 4-matmul groups between attention l-chunks; attention
           accumulated s-major; LN1 split across engines: residual add on
           Pool, bn_stats/normalize on DVE, rsqrt Newton chain on Pool.
  phase 2: per 128-row chunk: PE-transpose res (bf16), FF matmul,
           relu+residual split Pool/DVE, LN2, DMA out in halves.
"""
import numpy as np
import ml_dtypes
from contextlib import ExitStack

import concourse.bass as bass
import concourse.tile as tile
from concourse import bacc, mybir
from concourse.bass_utils import run_bass_kernel_spmd
from concourse.alu_op_type import AluOpType

F32 = mybir.dt.float32
BF16 = mybir.dt.bfloat16
I32 = mybir.dt.int32
ACTF = mybir.ActivationFunctionType

B, S, D, K = 8, 2048, 1024, 64
EPS = 1e-5
NCORES = 8
SB = 512          # superblock width (scores free dim)
NSB = S // SB     # 4
NC = S // 128     # 16 s-chunks
ND2 = D // 512    # 2 d-tiles


def build_program(flags):
    have_bkq, have_bb, have_bv, have_b1, have_gb = flags
    nc = bacc.Bacc(trn_type="TRN2")

    xt_d = nc.declare_dram_parameter("xt", [128, 8, S], BF16, isOutput=False)
    x_d = nc.declare_dram_parameter("x", [S, D], BF16, isOutput=False)
    wkq_d = nc.declare_dram_parameter("wkq", [128, 8, 128], BF16, isOutput=False)
    wb_d = nc.declare_dram_parameter("wb", [128, 8, 1], BF16, isOutput=False)
    wv_d = nc.declare_dram_parameter("wv", [128, 8, D], BF16, isOutput=False)
    w1_d = nc.declare_dram_parameter("w1", [128, 8, D], BF16, isOutput=False)
    ones_d = nc.declare_dram_parameter("onesrow", [1, S], BF16, isOutput=False)
    iden_d = nc.declare_dram_parameter("iden", [128, 128], BF16, isOutput=False)
    g1_d = nc.declare_dram_parameter("g1", [1, D], F32, isOutput=False)
    be1_d = nc.declare_dram_parameter("be1", [1, D], F32, isOutput=False)
    bkq_d = nc.declare_dram_parameter("bkq", [1, 128], BF16, isOutput=False)
    bb_d = nc.declare_dram_parameter("bb", [1, 1], BF16, isOutput=False)
    bv_d = nc.declare_dram_parameter("bv", [1, D], BF16, isOutput=False)
    b1_d = nc.declare_dram_parameter("b1", [1, D], BF16, isOutput=False)
    out_d = nc.declare_dram_parameter("out", [S, D], F32, isOutput=True)

    with tile.TileContext(nc) as tc, ExitStack() as top:
        const = top.enter_context(tc.tile_pool(name="const", bufs=1))
        kqp = top.enter_context(tc.tile_pool(name="kqp", bufs=1))
        vp = top.enter_context(tc.tile_pool(name="vp", bufs=1))
        resp = top.enter_context(tc.tile_pool(name="resp", bufs=1))
        w1p = top.enter_context(tc.tile_pool(name="w1p", bufs=1))

        # ---- constants (rsqrt chain runs on the Pool engine)
        eps_t = const.tile([128, 1], F32)
        nc.gpsimd.memset(eps_t, EPS)
        zero_t = const.tile([128, 1], F32)
        nc.gpsimd.memset(zero_t, 0.0)
        magic_t = const.tile([128, 1], I32)
        nc.gpsimd.memset(magic_t, 0x5f3759df)
        one_i = const.tile([128, 1], I32)
        nc.gpsimd.memset(one_i, 1)
        neghalf_t = const.tile([128, 1], F32)
        nc.gpsimd.memset(neghalf_t, -0.5)
        threehalf_t = const.tile([128, 1], F32)
        nc.gpsimd.memset(threehalf_t, 1.5)

        def emit_rstd(eng, pool, var_ap, nm, iters=2):
            """1/sqrt(var+EPS) via bit trick + Newton iters on `eng`.

            Pool-engine safe: only TensorTensor ops (no TensorScalarPtr).
            """
            scr = pool.tile([128, 5], F32, tag="scr", name=f"scr{nm}")
            vpe, y = scr[:, 0:1], scr[:, 1:2]
            y2, bt = scr[:, 2:3], scr[:, 3:4]
            dd = scr[:, 4:5]
            # int ops on DVE (Pool shifts need int64); the rest on `eng`
            nc.vector.tensor_tensor(vpe, var_ap, eps_t, op=AluOpType.add)
            nc.vector.tensor_tensor(out=y.bitcast(I32), in0=vpe.bitcast(I32),
                                    in1=one_i,
                                    op=AluOpType.logical_shift_right)
            nc.vector.tensor_tensor(out=y.bitcast(I32), in0=magic_t,
                                    in1=y.bitcast(I32), op=AluOpType.subtract)
            for _ in range(iters):
                eng.tensor_tensor(y2, y, y, op=AluOpType.mult)
                eng.tensor_tensor(bt, vpe, y2, op=AluOpType.mult)
                eng.tensor_tensor(dd, bt, neghalf_t, op=AluOpType.mult)
                eng.tensor_tensor(dd, dd, threehalf_t, op=AluOpType.add)
                eng.tensor_tensor(y, y, dd, op=AluOpType.mult)
            return y

        def emit_norm_consts(eng, pool, mv, nm):
            """rstd = 1/sqrt(var+eps); negmb = -mean*rstd (for activation
            Identity: out = in*rstd + negmb = (in - mean)*rstd)."""
            rstd = emit_rstd(eng, pool, mv[:, 1:2], nm)
            nb = pool.tile([128, 1], F32, tag="nb", name=f"nb{nm}")
            eng.tensor_tensor(nb, mv[:, 0:1], rstd, op=AluOpType.mult)
            eng.tensor_tensor(nb, zero_t, nb, op=AluOpType.subtract)
            return rstd, nb

        ones_t = const.tile([1, S], BF16)
        iden_t = const.tile([128, 128], BF16)
        if have_gb:
            g1_b = const.tile([128, D], F32)
            be1_b = const.tile([128, D], F32)
        if have_bkq:
            bkq_t = const.tile([1, 128], BF16)
        if have_bb:
            bb_t = const.tile([1, 1], BF16)
        if have_bv:
            bv_t = const.tile([1, D], BF16)
        if have_b1:
            b1_t = const.tile([1, D], BF16)

        def emit_const_dmas():
            # emitted after the critical xt/wv DMAs: these are small,
            # packet-inefficient transfers that must not delay them
            nc.scalar.dma_start(ones_t, ones_d.ap())
            nc.scalar.dma_start(iden_t, iden_d.ap())
            nc.scalar.dma_start(tile_q[64:65, :], ones_d.ap())
            if have_gb:
                nc.scalar.dma_start(g1_b, g1_d.ap().partition_broadcast(128))
                nc.scalar.dma_start(be1_b,
                                    be1_d.ap().partition_broadcast(128))
            if have_bkq:
                nc.scalar.dma_start(bkq_t, bkq_d.ap())
            if have_bb:
                nc.scalar.dma_start(bb_t, bb_d.ap())
            if have_bv:
                nc.scalar.dma_start(bv_t, bv_d.ap())
            if have_b1:
                nc.scalar.dma_start(b1_t, b1_d.ap())

        # ---- kq/bias output tiles (rows 0..64)
        tile_k = kqp.tile([65, S], BF16)   # rows0-63 scaled keysT, row64 biasesT
        tile_q = kqp.tile([65, S], BF16)   # rows0-63 queriesT, row64 ones

        v_sb = vp.tile([128, NC, D], BF16)      # values s-major
        res_sb = resp.tile([128, NC, D], BF16)  # LN1 output, SBUF resident
        w1_t = w1p.tile([128, 8, D], BF16)

        strip_tiles = [[None] * 4 for _ in range(NSB)]

        with ExitStack() as ph01:
            strips = ph01.enter_context(tc.tile_pool(name="strips", bufs=8))
            scores_env = {}

            def emit_scores(sb, jlist):
                ps_s = scores_env['ps_s']
                isl = slice(sb * SB, (sb + 1) * SB)
                for j in jlist:
                    if j % 4 == 0:
                        strip_tiles[sb][j // 4] = strips.tile(
                            [128, 4, SB], BF16, tag="strip",
                            name=f"strip_{sb}_{j // 4}")
                    pst = ps_s.tile([128, SB], F32, tag="s")
                    nc.tensor.matmul(
                        pst, tile_q[:, j * 128:(j + 1) * 128],
                        tile_k[:, isl], start=True, stop=True)
                    nc.scalar.activation(
                        strip_tiles[sb][j // 4][:, j % 4, :], pst,
                        ACTF.Sigmoid)

            # ================= phase 0: projections =================
            with ExitStack() as ph0:
                xp = ph0.enter_context(tc.tile_pool(name="xp", bufs=1))
                wp = ph0.enter_context(tc.tile_pool(name="wp", bufs=1))

                wkq_t = xp.tile([128, 8, 128], BF16)
                wb_t = xp.tile([128, 8, 1], BF16)
                xt_t = xp.tile([128, 8, S], BF16)
                wv_t = wp.tile([128, 8, D], BF16, tag="wv")
                # per-chunk half-split across the two HWDGE queues so
                # chunk k's data lands just in time for its kq matmuls
                # (the kq pass consumes chunks in order at ~1.7us each,
                # matching the two queues' ~0.3 MB/us aggregate rate);
                # wkq chunk 0 jumps the scalar queue so the first
                # LDWEIGHTS isn't gated on the full weight transfer
                nc.scalar.dma_start(wkq_t[:, 0, :], wkq_d[:, 0, :])
                nc.scalar.dma_start(xt_t[:, 0, D:S], xt_d[:, 0, D:S])
                nc.scalar.dma_start(wkq_t[:, 1:8, :], wkq_d[:, 1:8, :])
                nc.sync.dma_start(wb_t, wb_d.ap())
                for k in range(8):
                    nc.sync.dma_start(xt_t[:, k, 0:D], xt_d[:, k, 0:D])
                    if 0 < k < 6:
                        nc.scalar.dma_start(xt_t[:, k, D:S],
                                            xt_d[:, k, D:S])
                # late b-halves + wv on the SWDGE ring, gated on xt chunk
                # 2's arrival so they don't steal HBM bandwidth from the
                # kq-critical early chunks
                gate_t = const.tile([128, 1], BF16)
                nc.gpsimd.tensor_tensor(gate_t, xt_t[:, 2, 0:1],
                                        xt_t[:, 2, 0:1], op=AluOpType.mult)
                nc.gpsimd.dma_start(xt_t[:, 6, D:S], xt_d[:, 6, D:S])
                nc.gpsimd.dma_start(xt_t[:, 7, D:S], xt_d[:, 7, D:S])
                for k in range(8):
                    nc.gpsimd.dma_start(wv_t[:, k, :], wv_d[:, k, :])
                emit_const_dmas()

                # kq + biases in a single k-pass (paced to the xt DMA):
                # 4 kq banks + 2 bias banks
                with ExitStack() as ph0a:
                    pkq = ph0a.enter_context(
                        tc.tile_pool(name="pkq", bufs=4, space="PSUM"))
                    pb = ph0a.enter_context(
                        tc.tile_pool(name="pb", bufs=2, space="PSUM"))
                    warm = ph0a.enter_context(
                        tc.tile_pool(name="warm", bufs=1, space="PSUM"))
                    # PE warm-up: the HAM clock gate needs ~3.4us of
                    # sustained matmul activity to lift the 1.2 GHz cold
                    # throttle.  Fill the DMA-startup window (~1-10us)
                    # with inert zero matmuls so the kq pass runs at
                    # 2.4 GHz the moment xt chunk 0 lands.
                    warm_src = xp.tile([128, 512], BF16, tag="wsrc")
                    nc.vector.memset(warm_src, 0.0)
                    warm_ps = warm.tile([128, 512], F32, tag="wps")
                    for _ in range(4):
                        nc.tensor.matmul(warm_ps, warm_src[:, 0:128],
                                         warm_src, start=True, stop=True)
                    pks = {st: pkq.tile([128, 512], F32, tag="kq",
                                        name=f"pk{st}") for st in range(4)}
                    # bias rows: base partition must be 0/32/64, so three
                    # rows share one bank and the fourth gets its own
                    pbt0 = pb.tile([65, 512], F32, tag="b", name="pb0")
                    pbt1 = pb.tile([1, 512], F32, tag="b", name="pb1")

                    def pb_row(st):
                        return (pbt0[32 * st:32 * st + 1, :] if st < 3
                                else pbt1)
                    for k in range(7):
                        for st in range(4):
                            sl = slice(st * SB, (st + 1) * SB)
                            nc.tensor.matmul(
                                pks[st], wkq_t[:, k, :], xt_t[:, k, sl],
                                start=(k == 0), stop=False)
                        for st in range(4):
                            sl = slice(st * SB, (st + 1) * SB)
                            nc.tensor.matmul(
                                pb_row(st), wb_t[:, k, :], xt_t[:, k, sl],
                                start=(k == 0), stop=False)
                    # last k-chunk per-st so st0's copies overlap st1-3's
                    # matmuls; copies balanced across scalar and vector
                    for st in range(4):
                        sl = slice(st * SB, (st + 1) * SB)
                        nc.tensor.matmul(pks[st], wkq_t[:, 7, :],
                                         xt_t[:, 7, sl],
                                         start=False, stop=not have_bkq)
                        nc.tensor.matmul(pb_row(st), wb_t[:, 7, :],
                                         xt_t[:, 7, sl],
                                         start=False, stop=not have_bb)
                        if have_bkq:
                            nc.tensor.matmul(pks[st], bkq_t, ones_t[:, sl],
                                             start=False, stop=True)
                        if have_bb:
                            nc.tensor.matmul(pb_row(st), bb_t,
                                             ones_t[:, sl],
                                             start=False, stop=True)
                        if st % 2 == 0:
                            nc.scalar.activation(tile_k[0:64, sl],
                                                 pks[st][0:64, :], ACTF.Copy)
                            nc.vector.tensor_copy(tile_q[0:64, sl],
                                                  pks[st][64:128, :])
                        else:
                            nc.vector.tensor_copy(tile_k[0:64, sl],
                                                  pks[st][0:64, :])
                            nc.scalar.activation(tile_q[0:64, sl],
                                                 pks[st][64:128, :],
                                                 ACTF.Copy)
                        nc.scalar.activation(tile_k[64:65, sl], pb_row(st),
                                             ACTF.Copy)

                # values: out[s-chunk,128 x d-512], stationary xT slices
                scores_env['ps_s'] = ph01.enter_context(
                    tc.tile_pool(name="ps_s", bufs=4, space="PSUM"))
                pv = ph0.enter_context(
                    tc.tile_pool(name="pv", bufs=4, space="PSUM"))

                def emit_values(sc):
                    pvt = [pv.tile([128, 512], F32, tag="v",
                                   name=f"pv{sc % 2}_{d2}")
                           for d2 in range(ND2)]
                    for k in range(8):
                        for dt2 in range(ND2):
                            nc.tensor.matmul(
                                pvt[dt2], xt_t[:, k, sc * 128:(sc + 1) * 128],
                                wv_t[:, k, dt2 * 512:(dt2 + 1) * 512],
                                start=(k == 0),
                                stop=(k == 7 and not have_bv))
                    for dt2 in range(ND2):
                        dsl = slice(dt2 * 512, (dt2 + 1) * 512)
                        if have_bv:
                            nc.tensor.matmul(pvt[dt2], ones_t[:, 0:128],
                                             bv_t[:, dsl],
                                             start=False, stop=True)
                        if (sc * ND2 + dt2) % 2 == 0:
                            nc.scalar.activation(
                                v_sb[:, sc, dsl], pvt[dt2], ACTF.Copy)
                        else:
                            nc.vector.tensor_copy(v_sb[:, sc, dsl], pvt[dt2])

                # scores(0) interleaved into the values tail in 4-matmul
                # groups so all 16 sigmoids drain before attention starts
                for sc in range(16):
                    if sc >= 8 and sc % 2 == 0:
                        emit_scores(0, range(2 * (sc - 8), 2 * (sc - 8) + 4))
                    emit_values(sc)

            # ================= phase 1: scores + attention + LN1 =========
            with ExitStack() as ph1:
                xin = ph1.enter_context(tc.tile_pool(name="xin", bufs=4))
                rpp = ph1.enter_context(tc.tile_pool(name="rpp", bufs=3))
                stat = ph1.enter_context(tc.tile_pool(name="stat", bufs=8))
                ps_a = ph1.enter_context(
                    tc.tile_pool(name="ps_a", bufs=4, space="PSUM"))

                x_tiles = {}

                def emit_xdma(c):
                    # on the SWDGE ring, behind wv -- keeps the HWDGE
                    # queues free for the phase-0 critical path
                    if c < NC:
                        xt_ = xin.tile([128, D], BF16, tag="x", name=f"x{c}")
                        nc.gpsimd.dma_start(xt_,
                                            x_d[c * 128:(c + 1) * 128, :])
                        x_tiles[c] = xt_

                emit_xdma(0)
                emit_xdma(1)
                for sb in range(NSB):
                    for l in range(4):
                        c = sb * 4 + l
                        if c == 8:
                            # w1 pushed mid-phase-1: after the early x_t
                            # chunks, well before phase 2 reads it
                            nc.gpsimd.dma_start(w1_t[:, 0:4, :],
                                                w1_d[:, 0:4, :])
                            nc.gpsimd.dma_start(w1_t[:, 4:8, :],
                                                w1_d[:, 4:8, :])
                        emit_xdma(c + 2)
                        if sb < NSB - 1:
                            # next superblock's scores, 4 at a time, so the
                            # sigmoids hide under this l-chunk's attention
                            emit_scores(sb + 1, range(4 * l, 4 * l + 4))
                        pa = [ps_a.tile([128, 512], F32, tag="a",
                                        name=f"pa{c % 2}_{d2}")
                              for d2 in range(ND2)]
                        for j in range(NC):
                            st_tile = strip_tiles[sb][j // 4]
                            for dt2 in range(ND2):
                                nc.tensor.matmul(
                                    pa[dt2],
                                    st_tile[:, j % 4, l * 128:(l + 1) * 128],
                                    v_sb[:, j, dt2 * 512:(dt2 + 1) * 512],
                                    start=(j == 0), stop=(j == NC - 1))
                        x_t = x_tiles[c]
                        rp = rpp.tile([128, D], F32, tag="rp",
                                      name=f"rp{c % 3}")
                        for dt2 in range(ND2):
                            dsl = slice(dt2 * 512, (dt2 + 1) * 512)
                            nc.vector.tensor_tensor(
                                rp[:, dsl], pa[dt2], x_t[:, dsl],
                                op=AluOpType.add)
                        st_t = stat.tile([128, 2, 6], F32, tag="bst",
                                         name=f"bst{c}")
                        for g in range(2):
                            nc.vector.bn_stats(st_t[:, g, :],
                                               rp[:, g * 512:(g + 1) * 512])
                        mv = stat.tile([128, 2], F32, tag="mv", name=f"mv{c}")
                        nc.vector.bn_aggr(mv, st_t)
                        rstd, negmb = emit_norm_consts(nc.gpsimd, stat, mv,
                                                       f"r1_{c}")
                        if have_gb:
                            t1 = rpp.tile([128, D], F32, tag="t1",
                                          name=f"t1{c % 3}")
                            nc.vector.scalar_tensor_tensor(
                                out=t1, in0=rp, scalar=mv[:, 0:1], in1=g1_b,
                                op0=AluOpType.subtract, op1=AluOpType.mult)
                            nc.vector.scalar_tensor_tensor(
                                out=res_sb[:, c, :], in0=t1, scalar=rstd,
                                in1=be1_b,
                                op0=AluOpType.mult, op1=AluOpType.add)
                        else:
                            nc.scalar.activation(
                                res_sb[:, c, :], rp, ACTF.Identity,
                                bias=negmb, scale=rstd)

        # ================= phase 2: FF + LN2 =================
        with ExitStack() as ph2:
            rtp = ph2.enter_context(tc.tile_pool(name="rtp", bufs=4))
            f2 = ph2.enter_context(tc.tile_pool(name="f2", bufs=3))
            ostage = ph2.enter_context(tc.tile_pool(name="ostage", bufs=3))
            stat2 = ph2.enter_context(tc.tile_pool(name="stat2", bufs=8))
            ps_f = ph2.enter_context(
                tc.tile_pool(name="ps_f", bufs=4, space="PSUM"))

            rT = [None] * NC

            def stage_tr(c):
                # res chunk transposed via the DMA XBAR (SBUF->SBUF, one
                # descriptor) instead of 8 PE transposes + PSUM copies
                rt_ = rtp.tile([128, 8, 128], BF16, tag="rT",
                               name=f"rT{c % 4}")
                nc.sync.dma_start(rt_, res_sb[:, c, :], transpose=True)
                rT[c] = rt_

            def stage_ff(c):
                pf = [ps_f.tile([128, 512], F32, tag="f",
                                name=f"pf{c % 2}_{d2}") for d2 in range(ND2)]
                r2 = f2.tile([128, D], F32, tag="r2", name=f"r2{c % 3}")
                st_t = stat2.tile([128, 2, 6], F32, tag="bst2",
                                  name=f"bst2_{c}")
                last = (c == NC - 1)
                if last:
                    # half-outer so the first half's LN2 work overlaps the
                    # second half's matmuls -- shortens the kernel tail
                    for dt2 in range(ND2):
                        dsl = slice(dt2 * 512, (dt2 + 1) * 512)
                        for k in range(8):
                            nc.tensor.matmul(
                                pf[dt2], rT[c][:, k, :], w1_t[:, k, dsl],
                                start=(k == 0),
                                stop=(k == 7 and not have_b1))
                        if have_b1:
                            nc.tensor.matmul(pf[dt2], ones_t[:, 0:128],
                                             b1_t[:, dsl],
                                             start=False, stop=True)
                        nc.vector.scalar_tensor_tensor(
                            out=r2[:, dsl], in0=pf[dt2], scalar=zero_t,
                            in1=res_sb[:, c, dsl],
                            op0=AluOpType.max, op1=AluOpType.add)
                        nc.vector.bn_stats(st_t[:, dt2, :], r2[:, dsl])
                else:
                    for k in range(8):
                        for dt2 in range(ND2):
                            nc.tensor.matmul(
                                pf[dt2], rT[c][:, k, :],
                                w1_t[:, k, dt2 * 512:(dt2 + 1) * 512],
                                start=(k == 0),
                                stop=(k == 7 and not have_b1))
                    for dt2 in range(ND2):
                        dsl = slice(dt2 * 512, (dt2 + 1) * 512)
                        if have_b1:
                            nc.tensor.matmul(pf[dt2], ones_t[:, 0:128],
                                             b1_t[:, dsl],
                                             start=False, stop=True)
                        nc.vector.scalar_tensor_tensor(
                            out=r2[:, dsl], in0=pf[dt2], scalar=zero_t,
                            in1=res_sb[:, c, dsl],
                            op0=AluOpType.max, op1=AluOpType.add)
                    for g in range(2):
                        nc.vector.bn_stats(st_t[:, g, :],
                                           r2[:, g * 512:(g + 1) * 512])
                mv = stat2.tile([128, 2], F32, tag="mv2", name=f"mv2_{c}")
                nc.vector.bn_aggr(mv, st_t)
                o_t = ostage.tile([128, D], F32, tag="o", name=f"o{c % 3}")
                if c == NC - 1:
                    # last chunk: keep the whole chain on DVE (no
                    # cross-engine hops) so the kernel tail is short;
                    # runs in parallel with chunk NC-2's Pool chain
                    rstd = emit_rstd(nc.vector, stat2, mv[:, 1:2],
                                     f"r2_{c}", iters=1)
                    for h in range(4):
                        hsl = slice(h * 256, (h + 1) * 256)
                        nc.vector.tensor_scalar(
                            out=o_t[:, hsl], in0=r2[:, hsl],
                            scalar1=mv[:, 0:1], scalar2=rstd,
                            op0=AluOpType.subtract, op1=AluOpType.mult)
                        deng = nc.sync if h % 2 == 0 else nc.scalar
                        deng.dma_start(out_d[c * 128:(c + 1) * 128, hsl],
                                       o_t[:, hsl])
                else:
                    rstd, negmb = emit_norm_consts(nc.gpsimd, stat2, mv,
                                                   f"r2_{c}")
                    for h in range(2):
                        hsl = slice(h * 512, (h + 1) * 512)
                        nc.scalar.activation(
                            o_t[:, hsl], r2[:, hsl], ACTF.Identity,
                            bias=negmb, scale=rstd)
                        deng = nc.sync if h == 0 else nc.scalar
                        deng.dma_start(out_d[c * 128:(c + 1) * 128, hsl],
                                       o_t[:, hsl])

            for c in range(NC + 2):
                if c < NC:
                    stage_tr(c)
                if c >= 2:
                    stage_ff(c - 2)

    nc.finalize()
    return nc


_PROGRAM_CACHE = {}


def _get_program(flags):
    if flags not in _PROGRAM_CACHE:
        _PROGRAM_CACHE[flags] = build_program(flags)
    return _PROGRAM_CACHE[flags]


def kernel(x, Wk, bk, Wq, bq, Wv, bv, Wb, bb, W1, b1, g1, be1):
    import math
    scale = 1.0 / math.sqrt(K)
    flags = (
        bool(np.any(bk) or np.any(bq)),
        bool(np.any(bb)),
        bool(np.any(bv)),
        bool(np.any(b1)),
        bool(np.any(np.asarray(g1) != 1.0) or np.any(be1)),
    )
    nc = _get_program(flags)
    bf = ml_dtypes.bfloat16

    def _prep(w):
        # [D, M] -> [128, 8, M] matching SBUF [partition, chunk, free]
        return np.ascontiguousarray(
            np.asarray(w, np.float32).reshape(8, 128, -1).transpose(1, 0, 2)
        ).astype(bf)

    wkq = _prep(np.concatenate(
        [np.asarray(Wk, np.float32) * scale, np.asarray(Wq, np.float32)],
        axis=1))
    wb_ = _prep(Wb)
    wv_ = _prep(Wv)
    w1_ = _prep(W1)
    onesrow = np.ones((1, S), dtype=bf)
    iden = np.eye(128, dtype=np.float32).astype(bf)
    bkq = np.concatenate(
        [np.asarray(bk, np.float32) * scale,
         np.asarray(bq, np.float32)])[None, :].astype(bf)
    bbr = np.asarray(bb, np.float32).reshape(1, 1).astype(bf)
    bvr = np.asarray(bv, np.float32)[None, :].astype(bf)
    b1r = np.asarray(b1, np.float32)[None, :].astype(bf)
    g1r = np.asarray(g1, np.float32)[None, :]
    be1r = np.asarray(be1, np.float32)[None, :]

    xf = np.asarray(x, np.float32)
    xt = np.ascontiguousarray(np.transpose(xf, (0, 2, 1)))      # [B,D,S]
    xt = np.ascontiguousarray(
        xt.reshape(B, 8, 128, S).transpose(0, 2, 1, 3)).astype(bf)
    xs = xf.astype(bf)                                          # [B,S,D]

    in_maps = []
    for b in range(B):
        in_maps.append(dict(
            xt=xt[b], x=xs[b], wkq=wkq, wb=wb_, wv=wv_, w1=w1_,
            onesrow=onesrow, iden=iden, g1=g1r, be1=be1r, bkq=bkq, bb=bbr,
            bv=bvr, b1=b1r))

    res = run_bass_kernel_spmd(nc, in_maps, list(range(NCORES)), trace=False)
    out = np.stack([res.results[b]["out"] for b in range(B)], axis=0)
    return out.astype(np.float32)


# revision 42
# speedup vs baseline: 1.0728x; 1.0728x over previous
"""AttentionEncoder TRN2 Bass kernel (bf16 matmul path).

Data-parallel over batch: B=8 samples -> 8 NeuronCores, one sample per core.
All matmul operands are bf16 (host-cast, free); PSUM accumulation stays fp32.
res (LN1 output) stays resident in SBUF as bf16 -- no DRAM spill/reload.

Per-core pipeline (S=2048, D=1024, K=64):
  phase 0: kqT+biasT projections streamed against the xt DMA (chunk k of the
           contraction only needs xt[:,k,:]), then values s-major with
           stationary xT slices; scores(sb=0) interleaved into the values
           tail so the sigmoid pipeline fills for free.
  phase 1: per 512-col superblock: scoresT = sigmoid(qk+bias) via a 65-row
           contraction (row 64 = biasesT x ones), emitted one superblock
           ahead in 4-matmul groups between attention l-chunks; attention
           accumulated s-major; LN1 split across engines: residual add on
           Pool, bn_stats/normalize on DVE, rsqrt Newton chain on Pool.
  phase 2: per 128-row chunk: PE-transpose res (bf16), FF matmul,
           relu+residual split Pool/DVE, LN2, DMA out in halves.
"""
import numpy as np
import ml_dtypes
from contextlib import ExitStack

import concourse.bass as bass
import concourse.tile as tile
from concourse import bacc, mybir
from concourse.bass_utils import run_bass_kernel_spmd
from concourse.alu_op_type import AluOpType

F32 = mybir.dt.float32
BF16 = mybir.dt.bfloat16
I32 = mybir.dt.int32
ACTF = mybir.ActivationFunctionType

B, S, D, K = 8, 2048, 1024, 64
EPS = 1e-5
NCORES = 8
SB = 512          # superblock width (scores free dim)
NSB = S // SB     # 4
NC = S // 128     # 16 s-chunks
ND2 = D // 512    # 2 d-tiles


def build_program(flags):
    have_bkq, have_bb, have_bv, have_b1, have_gb = flags
    nc = bacc.Bacc(trn_type="TRN2")

    xt_d = nc.declare_dram_parameter("xt", [128, 8, S], BF16, isOutput=False)
    x_d = nc.declare_dram_parameter("x", [S, D], BF16, isOutput=False)
    wkq_d = nc.declare_dram_parameter("wkq", [128, 8, 128], BF16, isOutput=False)
    wb_d = nc.declare_dram_parameter("wb", [128, 8, 1], BF16, isOutput=False)
    wv_d = nc.declare_dram_parameter("wv", [128, 8, D], BF16, isOutput=False)
    w1_d = nc.declare_dram_parameter("w1", [128, 8, D], BF16, isOutput=False)
    ones_d = nc.declare_dram_parameter("onesrow", [1, S], BF16, isOutput=False)
    iden_d = nc.declare_dram_parameter("iden", [128, 128], BF16, isOutput=False)
    g1_d = nc.declare_dram_parameter("g1", [1, D], F32, isOutput=False)
    be1_d = nc.declare_dram_parameter("be1", [1, D], F32, isOutput=False)
    bkq_d = nc.declare_dram_parameter("bkq", [1, 128], BF16, isOutput=False)
    bb_d = nc.declare_dram_parameter("bb", [1, 1], BF16, isOutput=False)
    bv_d = nc.declare_dram_parameter("bv", [1, D], BF16, isOutput=False)
    b1_d = nc.declare_dram_parameter("b1", [1, D], BF16, isOutput=False)
    out_d = nc.declare_dram_parameter("out", [S, D], F32, isOutput=True)

    with tile.TileContext(nc) as tc, ExitStack() as top:
        const = top.enter_context(tc.tile_pool(name="const", bufs=1))
        kqp = top.enter_context(tc.tile_pool(name="kqp", bufs=1))
        vp = top.enter_context(tc.tile_pool(name="vp", bufs=1))
        resp = top.enter_context(tc.tile_pool(name="resp", bufs=1))
        w1p = top.enter_context(tc.tile_pool(name="w1p", bufs=1))

        # ---- constants (rsqrt chain runs on the Pool engine)
        eps_t = const.tile([128, 1], F32)
        nc.gpsimd.memset(eps_t, EPS)
        zero_t = const.tile([128, 1], F32)
        nc.gpsimd.memset(zero_t, 0.0)
        magic_t = const.tile([128, 1], I32)
        nc.gpsimd.memset(magic_t, 0x5f3759df)
        one_i = const.tile([128, 1], I32)
        nc.gpsimd.memset(one_i, 1)
        neghalf_t = const.tile([128, 1], F32)
        nc.gpsimd.memset(neghalf_t, -0.5)
        threehalf_t = const.tile([128, 1], F32)
        nc.gpsimd.memset(threehalf_t, 1.5)

        def emit_rstd(eng, pool, var_ap, nm, iters=2):
            """1/sqrt(var+EPS) via bit trick + Newton iters on `eng`.

            Pool-engine safe: only TensorTensor ops (no TensorScalarPtr).
            """
            scr = pool.tile([128, 5], F32, tag="scr", name=f"scr{nm}")
            vpe, y = scr[:, 0:1], scr[:, 1:2]
            y2, bt = scr[:, 2:3], scr[:, 3:4]
            dd = scr[:, 4:5]
            # int ops on DVE (Pool shifts need int64); the rest on `eng`
            nc.vector.tensor_tensor(vpe, var_ap, eps_t, op=AluOpType.add)
            nc.vector.tensor_tensor(out=y.bitcast(I32), in0=vpe.bitcast(I32),
                                    in1=one_i,
                                    op=AluOpType.logical_shift_right)
            nc.vector.tensor_tensor(out=y.bitcast(I32), in0=magic_t,
                                    in1=y.bitcast(I32), op=AluOpType.subtract)
            for _ in range(iters):
                eng.tensor_tensor(y2, y, y, op=AluOpType.mult)
                eng.tensor_tensor(bt, vpe, y2, op=AluOpType.mult)
                eng.tensor_tensor(dd, bt, neghalf_t, op=AluOpType.mult)
                eng.tensor_tensor(dd, dd, threehalf_t, op=AluOpType.add)
                eng.tensor_tensor(y, y, dd, op=AluOpType.mult)
            return y

        def emit_norm_consts(eng, pool, mv, nm):
            """rstd = 1/sqrt(var+eps); negmb = -mean*rstd (for activation
            Identity: out = in*rstd + negmb = (in - mean)*rstd)."""
            rstd = emit_rstd(eng, pool, mv[:, 1:2], nm)
            nb = pool.tile([128, 1], F32, tag="nb", name=f"nb{nm}")
            eng.tensor_tensor(nb, mv[:, 0:1], rstd, op=AluOpType.mult)
            eng.tensor_tensor(nb, zero_t, nb, op=AluOpType.subtract)
            return rstd, nb

        ones_t = const.tile([1, S], BF16)
        iden_t = const.tile([128, 128], BF16)
        if have_gb:
            g1_b = const.tile([128, D], F32)
            be1_b = const.tile([128, D], F32)
        if have_bkq:
            bkq_t = const.tile([1, 128], BF16)
        if have_bb:
            bb_t = const.tile([1, 1], BF16)
        if have_bv:
            bv_t = const.tile([1, D], BF16)
        if have_b1:
            b1_t = const.tile([1, D], BF16)

        def emit_const_dmas():
            # emitted after the critical xt/wv DMAs: these are small,
            # packet-inefficient transfers that must not delay them
            nc.scalar.dma_start(ones_t, ones_d.ap())
            nc.scalar.dma_start(iden_t, iden_d.ap())
            nc.scalar.dma_start(tile_q[64:65, :], ones_d.ap())
            if have_gb:
                nc.scalar.dma_start(g1_b, g1_d.ap().partition_broadcast(128))
                nc.scalar.dma_start(be1_b,
                                    be1_d.ap().partition_broadcast(128))
            if have_bkq:
                nc.scalar.dma_start(bkq_t, bkq_d.ap())
            if have_bb:
                nc.scalar.dma_start(bb_t, bb_d.ap())
            if have_bv:
                nc.scalar.dma_start(bv_t, bv_d.ap())
            if have_b1:
                nc.scalar.dma_start(b1_t, b1_d.ap())

        # ---- kq/bias output tiles (rows 0..64)
        tile_k = kqp.tile([65, S], BF16)   # rows0-63 scaled keysT, row64 biasesT
        tile_q = kqp.tile([65, S], BF16)   # rows0-63 queriesT, row64 ones

        v_sb = vp.tile([128, NC, D], BF16)      # values s-major
        res_sb = resp.tile([128, NC, D], BF16)  # LN1 output, SBUF resident
        w1_t = w1p.tile([128, 8, D], BF16)
        rtp = top.enter_context(tc.tile_pool(name="rtp", bufs=NC))
        rT = [None] * NC

        def emit_tr(c):
            # res chunk transposed via the DMA XBAR (SBUF->SBUF, one
            # descriptor), issued right after LN1 so the transfer hides
            # under phase 1; phase 2 then starts with resT ready
            rt_ = rtp.tile([128, 8, 128], BF16, tag="rT", name=f"rT{c}")
            eng = nc.sync if c % 2 == 0 else nc.scalar
            eng.dma_start(rt_, res_sb[:, c, :], transpose=True)
            rT[c] = rt_

        strip_tiles = [[None] * 4 for _ in range(NSB)]

        with ExitStack() as ph01:
            strips = ph01.enter_context(tc.tile_pool(name="strips", bufs=8))
            scores_env = {}

            def emit_scores(sb, jlist):
                ps_s = scores_env['ps_s']
                isl = slice(sb * SB, (sb + 1) * SB)
                for j in jlist:
                    if j % 4 == 0:
                        strip_tiles[sb][j // 4] = strips.tile(
                            [128, 4, SB], BF16, tag="strip",
                            name=f"strip_{sb}_{j // 4}")
                    pst = ps_s.tile([128, SB], F32, tag="s")
                    nc.tensor.matmul(
                        pst, tile_q[:, j * 128:(j + 1) * 128],
                        tile_k[:, isl], start=True, stop=True)
                    nc.scalar.activation(
                        strip_tiles[sb][j // 4][:, j % 4, :], pst,
                        ACTF.Sigmoid)

            # ================= phase 0: projections =================
            with ExitStack() as ph0:
                xp = ph0.enter_context(tc.tile_pool(name="xp", bufs=1))
                wp = ph0.enter_context(tc.tile_pool(name="wp", bufs=1))

                wkq_t = xp.tile([128, 8, 128], BF16)
                wb_t = xp.tile([128, 8, 1], BF16)
                xt_t = xp.tile([128, 8, S], BF16)
                wv_t = wp.tile([128, 8, D], BF16, tag="wv")
                # per-chunk half-split across the two HWDGE queues so
                # chunk k's data lands just in time for its kq matmuls
                # (the kq pass consumes chunks in order at ~1.7us each,
                # matching the two queues' ~0.3 MB/us aggregate rate);
                # wkq chunk 0 jumps the scalar queue so the first
                # LDWEIGHTS isn't gated on the full weight transfer
                nc.scalar.dma_start(wkq_t[:, 0, :], wkq_d[:, 0, :])
                nc.scalar.dma_start(xt_t[:, 0, D:S], xt_d[:, 0, D:S])
                nc.scalar.dma_start(wkq_t[:, 1:8, :], wkq_d[:, 1:8, :])
                nc.sync.dma_start(wb_t, wb_d.ap())
                for k in range(8):
                    nc.sync.dma_start(xt_t[:, k, 0:D], xt_d[:, k, 0:D])
                    if 0 < k < 6:
                        nc.scalar.dma_start(xt_t[:, k, D:S],
                                            xt_d[:, k, D:S])
                # late b-halves + wv on the SWDGE ring, gated on xt chunk
                # 2's arrival so they don't steal HBM bandwidth from the
                # kq-critical early chunks
                gate_t = const.tile([128, 1], BF16)
                nc.gpsimd.tensor_tensor(gate_t, xt_t[:, 2, 0:1],
                                        xt_t[:, 2, 0:1], op=AluOpType.mult)
                nc.gpsimd.dma_start(xt_t[:, 6, D:S], xt_d[:, 6, D:S])
                nc.gpsimd.dma_start(xt_t[:, 7, D:S], xt_d[:, 7, D:S])
                for k in range(8):
                    nc.gpsimd.dma_start(wv_t[:, k, :], wv_d[:, k, :])
                emit_const_dmas()

                # kq + biases in a single k-pass (paced to the xt DMA):
                # 4 kq banks + 2 bias banks
                with ExitStack() as ph0a:
                    pkq = ph0a.enter_context(
                        tc.tile_pool(name="pkq", bufs=4, space="PSUM"))
                    pb = ph0a.enter_context(
                        tc.tile_pool(name="pb", bufs=2, space="PSUM"))
                    warm = ph0a.enter_context(
                        tc.tile_pool(name="warm", bufs=1, space="PSUM"))
                    # PE warm-up: the HAM clock gate needs ~3.4us of
                    # sustained matmul activity to lift the 1.2 GHz cold
                    # throttle.  Fill the DMA-startup window (~1-10us)
                    # with inert zero matmuls so the kq pass runs at
                    # 2.4 GHz the moment xt chunk 0 lands.
                    warm_src = xp.tile([128, 512], BF16, tag="wsrc")
                    nc.vector.memset(warm_src, 0.0)
                    warm_ps = warm.tile([128, 512], F32, tag="wps")
                    for _ in range(4):
                        nc.tensor.matmul(warm_ps, warm_src[:, 0:128],
                                         warm_src, start=True, stop=True)
                    pks = {st: pkq.tile([128, 512], F32, tag="kq",
                                        name=f"pk{st}") for st in range(4)}
                    # bias rows: base partition must be 0/32/64, so three
                    # rows share one bank and the fourth gets its own
                    pbt0 = pb.tile([65, 512], F32, tag="b", name="pb0")
                    pbt1 = pb.tile([1, 512], F32, tag="b", name="pb1")

                    def pb_row(st):
                        return (pbt0[32 * st:32 * st + 1, :] if st < 3
                                else pbt1)
                    for k in range(7):
                        for st in range(4):
                            sl = slice(st * SB, (st + 1) * SB)
                            nc.tensor.matmul(
                                pks[st], wkq_t[:, k, :], xt_t[:, k, sl],
                                start=(k == 0), stop=False)
                        for st in range(4):
                            sl = slice(st * SB, (st + 1) * SB)
                            nc.tensor.matmul(
                                pb_row(st), wb_t[:, k, :], xt_t[:, k, sl],
                                start=(k == 0), stop=False)
                    # last k-chunk per-st so st0's copies overlap st1-3's
                    # matmuls; copies balanced across scalar and vector
                    for st in range(4):
                        sl = slice(st * SB, (st + 1) * SB)
                        nc.tensor.matmul(pks[st], wkq_t[:, 7, :],
                                         xt_t[:, 7, sl],
                                         start=False, stop=not have_bkq)
                        nc.tensor.matmul(pb_row(st), wb_t[:, 7, :],
                                         xt_t[:, 7, sl],
                                         start=False, stop=not have_bb)
                        if have_bkq:
                            nc.tensor.matmul(pks[st], bkq_t, ones_t[:, sl],
                                             start=False, stop=True)
                        if have_bb:
                            nc.tensor.matmul(pb_row(st), bb_t,
                                             ones_t[:, sl],
                                             start=False, stop=True)
                        if st % 2 == 0:
                            nc.scalar.activation(tile_k[0:64, sl],
                                                 pks[st][0:64, :], ACTF.Copy)
                            nc.vector.tensor_copy(tile_q[0:64, sl],
                                                  pks[st][64:128, :])
                        else:
                            nc.vector.tensor_copy(tile_k[0:64, sl],
                                                  pks[st][0:64, :])
                            nc.scalar.activation(tile_q[0:64, sl],
                                                 pks[st][64:128, :],
                                                 ACTF.Copy)
                        nc.scalar.activation(tile_k[64:65, sl], pb_row(st),
                                             ACTF.Copy)

                # values: out[s-chunk,128 x d-512], stationary xT slices
                scores_env['ps_s'] = ph01.enter_context(
                    tc.tile_pool(name="ps_s", bufs=4, space="PSUM"))
                pv = ph0.enter_context(
                    tc.tile_pool(name="pv", bufs=4, space="PSUM"))

                def emit_values(sc):
                    pvt = [pv.tile([128, 512], F32, tag="v",
                                   name=f"pv{sc % 2}_{d2}")
                           for d2 in range(ND2)]
                    for k in range(8):
                        for dt2 in range(ND2):
                            nc.tensor.matmul(
                                pvt[dt2], xt_t[:, k, sc * 128:(sc + 1) * 128],
                                wv_t[:, k, dt2 * 512:(dt2 + 1) * 512],
                                start=(k == 0),
                                stop=(k == 7 and not have_bv))
                    for dt2 in range(ND2):
                        dsl = slice(dt2 * 512, (dt2 + 1) * 512)
                        if have_bv:
                            nc.tensor.matmul(pvt[dt2], ones_t[:, 0:128],
                                             bv_t[:, dsl],
                                             start=False, stop=True)
                        if (sc * ND2 + dt2) % 2 == 0:
                            nc.scalar.activation(
                                v_sb[:, sc, dsl], pvt[dt2], ACTF.Copy)
                        else:
                            nc.vector.tensor_copy(v_sb[:, sc, dsl], pvt[dt2])

                # scores(0) interleaved into the values tail in 4-matmul
                # groups so all 16 sigmoids drain before attention starts
                for sc in range(16):
                    if sc >= 8 and sc % 2 == 0:
                        emit_scores(0, range(2 * (sc - 8), 2 * (sc - 8) + 4))
                    emit_values(sc)

            # ================= phase 1: scores + attention + LN1 =========
            with ExitStack() as ph1:
                xin = ph1.enter_context(tc.tile_pool(name="xin", bufs=4))
                rpp = ph1.enter_context(tc.tile_pool(name="rpp", bufs=3))
                stat = ph1.enter_context(tc.tile_pool(name="stat", bufs=8))
                ps_a = ph1.enter_context(
                    tc.tile_pool(name="ps_a", bufs=4, space="PSUM"))

                x_tiles = {}

                def emit_xdma(c):
                    # on the SWDGE ring, behind wv -- keeps the HWDGE
                    # queues free for the phase-0 critical path
                    if c < NC:
                        xt_ = xin.tile([128, D], BF16, tag="x", name=f"x{c}")
                        nc.gpsimd.dma_start(xt_,
                                            x_d[c * 128:(c + 1) * 128, :])
                        x_tiles[c] = xt_

                emit_xdma(0)
                emit_xdma(1)
                for sb in range(NSB):
                    for l in range(4):
                        c = sb * 4 + l
                        if c == 8:
                            # w1 pushed mid-phase-1: after the early x_t
                            # chunks, well before phase 2 reads it
                            nc.gpsimd.dma_start(w1_t[:, 0:4, :],
                                                w1_d[:, 0:4, :])
                            nc.gpsimd.dma_start(w1_t[:, 4:8, :],
                                                w1_d[:, 4:8, :])
                        emit_xdma(c + 2)
                        if sb < NSB - 1:
                            # next superblock's scores, 4 at a time, so the
                            # sigmoids hide under this l-chunk's attention
                            emit_scores(sb + 1, range(4 * l, 4 * l + 4))
                        pa = [ps_a.tile([128, 512], F32, tag="a",
                                        name=f"pa{c % 2}_{d2}")
                              for d2 in range(ND2)]
                        for j in range(NC):
                            st_tile = strip_tiles[sb][j // 4]
                            for dt2 in range(ND2):
                                nc.tensor.matmul(
                                    pa[dt2],
                                    st_tile[:, j % 4, l * 128:(l + 1) * 128],
                                    v_sb[:, j, dt2 * 512:(dt2 + 1) * 512],
                                    start=(j == 0), stop=(j == NC - 1))
                        x_t = x_tiles[c]
                        rp = rpp.tile([128, D], F32, tag="rp",
                                      name=f"rp{c % 3}")
                        for dt2 in range(ND2):
                            dsl = slice(dt2 * 512, (dt2 + 1) * 512)
                            nc.vector.tensor_tensor(
                                rp[:, dsl], pa[dt2], x_t[:, dsl],
                                op=AluOpType.add)
                        st_t = stat.tile([128, 2, 6], F32, tag="bst",
                                         name=f"bst{c}")
                        for g in range(2):
                            nc.vector.bn_stats(st_t[:, g, :],
                                               rp[:, g * 512:(g + 1) * 512])
                        mv = stat.tile([128, 2], F32, tag="mv", name=f"mv{c}")
                        nc.vector.bn_aggr(mv, st_t)
                        rstd, negmb = emit_norm_consts(nc.gpsimd, stat, mv,
                                                       f"r1_{c}")
                        if have_gb:
                            t1 = rpp.tile([128, D], F32, tag="t1",
                                          name=f"t1{c % 3}")
                            nc.vector.scalar_tensor_tensor(
                                out=t1, in0=rp, scalar=mv[:, 0:1], in1=g1_b,
                                op0=AluOpType.subtract, op1=AluOpType.mult)
                            nc.vector.scalar_tensor_tensor(
                                out=res_sb[:, c, :], in0=t1, scalar=rstd,
                                in1=be1_b,
                                op0=AluOpType.mult, op1=AluOpType.add)
                        else:
                            nc.scalar.activation(
                                res_sb[:, c, :], rp, ACTF.Identity,
                                bias=negmb, scale=rstd)
                        emit_tr(c)

        # ================= phase 2: FF + LN2 =================
        with ExitStack() as ph2:
            rtp = ph2.enter_context(tc.tile_pool(name="rtp", bufs=4))
            f2 = ph2.enter_context(tc.tile_pool(name="f2", bufs=3))
            ostage = ph2.enter_context(tc.tile_pool(name="ostage", bufs=3))
            stat2 = ph2.enter_context(tc.tile_pool(name="stat2", bufs=8))
            ps_f = ph2.enter_context(
                tc.tile_pool(name="ps_f", bufs=4, space="PSUM"))

            def stage_ff(c):
                pf = [ps_f.tile([128, 512], F32, tag="f",
                                name=f"pf{c % 2}_{d2}") for d2 in range(ND2)]
                r2 = f2.tile([128, D], F32, tag="r2", name=f"r2{c % 3}")
                st_t = stat2.tile([128, 2, 6], F32, tag="bst2",
                                  name=f"bst2_{c}")
                last = (c == NC - 1)
                if last:
                    # half-outer so the first half's LN2 work overlaps the
                    # second half's matmuls -- shortens the kernel tail
                    for dt2 in range(ND2):
                        dsl = slice(dt2 * 512, (dt2 + 1) * 512)
                        for k in range(8):
                            nc.tensor.matmul(
                                pf[dt2], rT[c][:, k, :], w1_t[:, k, dsl],
                                start=(k == 0),
                                stop=(k == 7 and not have_b1))
                        if have_b1:
                            nc.tensor.matmul(pf[dt2], ones_t[:, 0:128],
                                             b1_t[:, dsl],
                                             start=False, stop=True)
                        nc.vector.scalar_tensor_tensor(
                            out=r2[:, dsl], in0=pf[dt2], scalar=zero_t,
                            in1=res_sb[:, c, dsl],
                            op0=AluOpType.max, op1=AluOpType.add)
                        nc.vector.bn_stats(st_t[:, dt2, :], r2[:, dsl])
                else:
                    for k in range(8):
                        for dt2 in range(ND2):
                            nc.tensor.matmul(
                                pf[dt2], rT[c][:, k, :],
                                w1_t[:, k, dt2 * 512:(dt2 + 1) * 512],
                                start=(k == 0),
                                stop=(k == 7 and not have_b1))
                    for dt2 in range(ND2):
                        dsl = slice(dt2 * 512, (dt2 + 1) * 512)
                        if have_b1:
                            nc.tensor.matmul(pf[dt2], ones_t[:, 0:128],
                                             b1_t[:, dsl],
                                             start=False, stop=True)
                        nc.vector.scalar_tensor_tensor(
                            out=r2[:, dsl], in0=pf[dt2], scalar=zero_t,
                            in1=res_sb[:, c, dsl],
                            op0=AluOpType.max, op1=AluOpType.add)
                    for g in range(2):
                        nc.vector.bn_stats(st_t[:, g, :],
                                           r2[:, g * 512:(g + 1) * 512])
                mv = stat2.tile([128, 2], F32, tag="mv2", name=f"mv2_{c}")
                nc.vector.bn_aggr(mv, st_t)
                o_t = ostage.tile([128, D], F32, tag="o", name=f"o{c % 3}")
                if c == NC - 1:
                    # last chunk: keep the whole chain on DVE (no
                    # cross-engine hops) so the kernel tail is short;
                    # runs in parallel with chunk NC-2's Pool chain
                    rstd = emit_rstd(nc.vector, stat2, mv[:, 1:2],
                                     f"r2_{c}", iters=1)
                    for h in range(4):
                        hsl = slice(h * 256, (h + 1) * 256)
                        nc.vector.tensor_scalar(
                            out=o_t[:, hsl], in0=r2[:, hsl],
                            scalar1=mv[:, 0:1], scalar2=rstd,
                            op0=AluOpType.subtract, op1=AluOpType.mult)
                        deng = nc.sync if h % 2 == 0 else nc.scalar
                        deng.dma_start(out_d[c * 128:(c + 1) * 128, hsl],
                                       o_t[:, hsl])
                else:
                    rstd, negmb = emit_norm_consts(nc.gpsimd, stat2, mv,
                                                   f"r2_{c}")
                    for h in range(2):
                        hsl = slice(h * 512, (h + 1) * 512)
                        nc.scalar.activation(
                            o_t[:, hsl], r2[:, hsl], ACTF.Identity,
                            bias=negmb, scale=rstd)
                        deng = nc.sync if h == 0 else nc.scalar
                        deng.dma_start(out_d[c * 128:(c + 1) * 128, hsl],
                                       o_t[:, hsl])

            for c in range(NC):
                stage_ff(c)

    nc.finalize()
    return nc


_PROGRAM_CACHE = {}


def _get_program(flags):
    if flags not in _PROGRAM_CACHE:
        _PROGRAM_CACHE[flags] = build_program(flags)
    return _PROGRAM_CACHE[flags]


def kernel(x, Wk, bk, Wq, bq, Wv, bv, Wb, bb, W1, b1, g1, be1):
    import math
    scale = 1.0 / math.sqrt(K)
    flags = (
        bool(np.any(bk) or np.any(bq)),
        bool(np.any(bb)),
        bool(np.any(bv)),
        bool(np.any(b1)),
        bool(np.any(np.asarray(g1) != 1.0) or np.any(be1)),
    )
    nc = _get_program(flags)
    bf = ml_dtypes.bfloat16

    def _prep(w):
        # [D, M] -> [128, 8, M] matching SBUF [partition, chunk, free]
        return np.ascontiguousarray(
            np.asarray(w, np.float32).reshape(8, 128, -1).transpose(1, 0, 2)
        ).astype(bf)

    wkq = _prep(np.concatenate(
        [np.asarray(Wk, np.float32) * scale, np.asarray(Wq, np.float32)],
        axis=1))
    wb_ = _prep(Wb)
    wv_ = _prep(Wv)
    w1_ = _prep(W1)
    onesrow = np.ones((1, S), dtype=bf)
    iden = np.eye(128, dtype=np.float32).astype(bf)
    bkq = np.concatenate(
        [np.asarray(bk, np.float32) * scale,
         np.asarray(bq, np.float32)])[None, :].astype(bf)
    bbr = np.asarray(bb, np.float32).reshape(1, 1).astype(bf)
    bvr = np.asarray(bv, np.float32)[None, :].astype(bf)
    b1r = np.asarray(b1, np.float32)[None, :].astype(bf)
    g1r = np.asarray(g1, np.float32)[None, :]
    be1r = np.asarray(be1, np.float32)[None, :]

    xf = np.asarray(x, np.float32)
    xt = np.ascontiguousarray(np.transpose(xf, (0, 2, 1)))      # [B,D,S]
    xt = np.ascontiguousarray(
        xt.reshape(B, 8, 128, S).transpose(0, 2, 1, 3)).astype(bf)
    xs = xf.astype(bf)                                          # [B,S,D]

    in_maps = []
    for b in range(B):
        in_maps.append(dict(
            xt=xt[b], x=xs[b], wkq=wkq, wb=wb_, wv=wv_, w1=w1_,
            onesrow=onesrow, iden=iden, g1=g1r, be1=be1r, bkq=bkq, bb=bbr,
            bv=bvr, b1=b1r))

    res = run_bass_kernel_spmd(nc, in_maps, list(range(NCORES)), trace=False)
    out = np.stack([res.results[b]["out"] for b in range(B)], axis=0)
    return out.astype(np.float32)


# revision 43
# speedup vs baseline: 1.0858x; 1.0121x over previous
"""AttentionEncoder TRN2 Bass kernel (bf16 matmul path).

Data-parallel over batch: B=8 samples -> 8 NeuronCores, one sample per core.
All matmul operands are bf16 (host-cast, free); PSUM accumulation stays fp32.
res (LN1 output) stays resident in SBUF as bf16 -- no DRAM spill/reload.

Per-core pipeline (S=2048, D=1024, K=64):
  phase 0: kqT+biasT projections streamed against the xt DMA (chunk k of the
           contraction only needs xt[:,k,:]), then values s-major with
           stationary xT slices; scores(sb=0) interleaved into the values
           tail so the sigmoid pipeline fills for free.
  phase 1: per 512-col superblock: scoresT = sigmoid(qk+bias) via a 65-row
           contraction (row 64 = biasesT x ones), emitted one superblock
           ahead in 4-matmul groups between attention l-chunks; attention
           accumulated s-major; LN1 split across engines: residual add on
           Pool, bn_stats/normalize on DVE, rsqrt Newton chain on Pool.
  phase 2: per 128-row chunk: PE-transpose res (bf16), FF matmul,
           relu+residual split Pool/DVE, LN2, DMA out in halves.
"""
import numpy as np
import ml_dtypes
from contextlib import ExitStack

import concourse.bass as bass
import concourse.tile as tile
from concourse import bacc, mybir
from concourse.bass_utils import run_bass_kernel_spmd
from concourse.alu_op_type import AluOpType

F32 = mybir.dt.float32
BF16 = mybir.dt.bfloat16
I32 = mybir.dt.int32
ACTF = mybir.ActivationFunctionType

B, S, D, K = 8, 2048, 1024, 64
EPS = 1e-5
NCORES = 8
SB = 512          # superblock width (scores free dim)
NSB = S // SB     # 4
NC = S // 128     # 16 s-chunks
ND2 = D // 512    # 2 d-tiles


def build_program(flags):
    have_bkq, have_bb, have_bv, have_b1, have_gb = flags
    nc = bacc.Bacc(trn_type="TRN2")

    xt_d = nc.declare_dram_parameter("xt", [128, 8, S], BF16, isOutput=False)
    x_d = nc.declare_dram_parameter("x", [S, D], BF16, isOutput=False)
    wkq_d = nc.declare_dram_parameter("wkq", [128, 8, 128], BF16, isOutput=False)
    wb_d = nc.declare_dram_parameter("wb", [128, 8, 1], BF16, isOutput=False)
    wv_d = nc.declare_dram_parameter("wv", [128, 8, D], BF16, isOutput=False)
    w1_d = nc.declare_dram_parameter("w1", [128, 8, D], BF16, isOutput=False)
    ones_d = nc.declare_dram_parameter("onesrow", [1, S], BF16, isOutput=False)
    iden_d = nc.declare_dram_parameter("iden", [128, 128], BF16, isOutput=False)
    g1_d = nc.declare_dram_parameter("g1", [1, D], F32, isOutput=False)
    be1_d = nc.declare_dram_parameter("be1", [1, D], F32, isOutput=False)
    bkq_d = nc.declare_dram_parameter("bkq", [1, 128], BF16, isOutput=False)
    bb_d = nc.declare_dram_parameter("bb", [1, 1], BF16, isOutput=False)
    bv_d = nc.declare_dram_parameter("bv", [1, D], BF16, isOutput=False)
    b1_d = nc.declare_dram_parameter("b1", [1, D], BF16, isOutput=False)
    out_d = nc.declare_dram_parameter("out", [S, D], F32, isOutput=True)

    with tile.TileContext(nc) as tc, ExitStack() as top:
        const = top.enter_context(tc.tile_pool(name="const", bufs=1))
        kqp = top.enter_context(tc.tile_pool(name="kqp", bufs=1))
        vp = top.enter_context(tc.tile_pool(name="vp", bufs=1))
        resp = top.enter_context(tc.tile_pool(name="resp", bufs=1))
        w1p = top.enter_context(tc.tile_pool(name="w1p", bufs=1))

        # ---- constants (rsqrt chain runs on the Pool engine)
        eps_t = const.tile([128, 1], F32)
        nc.gpsimd.memset(eps_t, EPS)
        zero_t = const.tile([128, 1], F32)
        nc.gpsimd.memset(zero_t, 0.0)
        magic_t = const.tile([128, 1], I32)
        nc.gpsimd.memset(magic_t, 0x5f3759df)
        one_i = const.tile([128, 1], I32)
        nc.gpsimd.memset(one_i, 1)
        neghalf_t = const.tile([128, 1], F32)
        nc.gpsimd.memset(neghalf_t, -0.5)
        threehalf_t = const.tile([128, 1], F32)
        nc.gpsimd.memset(threehalf_t, 1.5)

        def emit_rstd(eng, pool, var_ap, nm, iters=2):
            """1/sqrt(var+EPS) via bit trick + Newton iters on `eng`.

            Pool-engine safe: only TensorTensor ops (no TensorScalarPtr).
            """
            scr = pool.tile([128, 5], F32, tag="scr", name=f"scr{nm}")
            vpe, y = scr[:, 0:1], scr[:, 1:2]
            y2, bt = scr[:, 2:3], scr[:, 3:4]
            dd = scr[:, 4:5]
            # int ops on DVE (Pool shifts need int64); the rest on `eng`
            nc.vector.tensor_tensor(vpe, var_ap, eps_t, op=AluOpType.add)
            nc.vector.tensor_tensor(out=y.bitcast(I32), in0=vpe.bitcast(I32),
                                    in1=one_i,
                                    op=AluOpType.logical_shift_right)
            nc.vector.tensor_tensor(out=y.bitcast(I32), in0=magic_t,
                                    in1=y.bitcast(I32), op=AluOpType.subtract)
            for _ in range(iters):
                eng.tensor_tensor(y2, y, y, op=AluOpType.mult)
                eng.tensor_tensor(bt, vpe, y2, op=AluOpType.mult)
                eng.tensor_tensor(dd, bt, neghalf_t, op=AluOpType.mult)
                eng.tensor_tensor(dd, dd, threehalf_t, op=AluOpType.add)
                eng.tensor_tensor(y, y, dd, op=AluOpType.mult)
            return y

        def emit_norm_consts(eng, pool, mv, nm):
            """rstd = 1/sqrt(var+eps); negmb = -mean*rstd (for activation
            Identity: out = in*rstd + negmb = (in - mean)*rstd)."""
            rstd = emit_rstd(eng, pool, mv[:, 1:2], nm)
            nb = pool.tile([128, 1], F32, tag="nb", name=f"nb{nm}")
            eng.tensor_tensor(nb, mv[:, 0:1], rstd, op=AluOpType.mult)
            eng.tensor_tensor(nb, zero_t, nb, op=AluOpType.subtract)
            return rstd, nb

        ones_t = const.tile([1, S], BF16)
        iden_t = const.tile([128, 128], BF16)
        if have_gb:
            g1_b = const.tile([128, D], F32)
            be1_b = const.tile([128, D], F32)
        if have_bkq:
            bkq_t = const.tile([1, 128], BF16)
        if have_bb:
            bb_t = const.tile([1, 1], BF16)
        if have_bv:
            bv_t = const.tile([1, D], BF16)
        if have_b1:
            b1_t = const.tile([1, D], BF16)

        def emit_const_dmas():
            # emitted after the critical xt/wv DMAs: these are small,
            # packet-inefficient transfers that must not delay them
            nc.scalar.dma_start(ones_t, ones_d.ap())
            nc.scalar.dma_start(iden_t, iden_d.ap())
            nc.scalar.dma_start(tile_q[64:65, :], ones_d.ap())
            if have_gb:
                nc.scalar.dma_start(g1_b, g1_d.ap().partition_broadcast(128))
                nc.scalar.dma_start(be1_b,
                                    be1_d.ap().partition_broadcast(128))
            if have_bkq:
                nc.scalar.dma_start(bkq_t, bkq_d.ap())
            if have_bb:
                nc.scalar.dma_start(bb_t, bb_d.ap())
            if have_bv:
                nc.scalar.dma_start(bv_t, bv_d.ap())
            if have_b1:
                nc.scalar.dma_start(b1_t, b1_d.ap())

        # ---- kq/bias output tiles (rows 0..64)
        tile_k = kqp.tile([65, S], BF16)   # rows0-63 scaled keysT, row64 biasesT
        tile_q = kqp.tile([65, S], BF16)   # rows0-63 queriesT, row64 ones

        v_sb = vp.tile([128, NC, D], BF16)      # values s-major
        res_sb = resp.tile([128, NC, D], BF16)  # LN1 output, SBUF resident
        w1_t = w1p.tile([128, 8, D], BF16)
        rtp = top.enter_context(tc.tile_pool(name="rtp", bufs=NC))
        rT = [None] * NC

        def emit_tr(c):
            # res chunk transposed via the DMA XBAR (SBUF->SBUF, one
            # descriptor), issued right after LN1 so the transfer hides
            # under phase 1; phase 2 then starts with resT ready
            rt_ = rtp.tile([128, 8, 128], BF16, tag="rT", name=f"rT{c}")
            eng = nc.sync if c % 2 == 0 else nc.scalar
            eng.dma_start(rt_, res_sb[:, c, :], transpose=True)
            rT[c] = rt_

        strip_tiles = [[None] * 4 for _ in range(NSB)]

        with ExitStack() as ph01:
            strips = ph01.enter_context(tc.tile_pool(name="strips", bufs=8))
            scores_env = {}

            def emit_scores(sb, jlist):
                ps_s = scores_env['ps_s']
                isl = slice(sb * SB, (sb + 1) * SB)
                for j in jlist:
                    if j % 4 == 0:
                        strip_tiles[sb][j // 4] = strips.tile(
                            [128, 4, SB], BF16, tag="strip",
                            name=f"strip_{sb}_{j // 4}")
                    pst = ps_s.tile([128, SB], F32, tag="s")
                    nc.tensor.matmul(
                        pst, tile_q[:, j * 128:(j + 1) * 128],
                        tile_k[:, isl], start=True, stop=True)
                    nc.scalar.activation(
                        strip_tiles[sb][j // 4][:, j % 4, :], pst,
                        ACTF.Sigmoid)

            # ================= phase 0: projections =================
            with ExitStack() as ph0:
                xp = ph0.enter_context(tc.tile_pool(name="xp", bufs=1))
                wp = ph0.enter_context(tc.tile_pool(name="wp", bufs=1))

                wkq_t = xp.tile([128, 8, 128], BF16)
                wb_t = xp.tile([128, 8, 1], BF16)
                xt_t = xp.tile([128, 8, S], BF16)
                wv_t = wp.tile([128, 8, D], BF16, tag="wv")
                # per-chunk half-split across the two HWDGE queues so
                # chunk k's data lands just in time for its kq matmuls
                # (the kq pass consumes chunks in order at ~1.7us each,
                # matching the two queues' ~0.3 MB/us aggregate rate);
                # wkq chunk 0 jumps the scalar queue so the first
                # LDWEIGHTS isn't gated on the full weight transfer
                nc.scalar.dma_start(wkq_t[:, 0, :], wkq_d[:, 0, :])
                nc.scalar.dma_start(xt_t[:, 0, D:S], xt_d[:, 0, D:S])
                nc.gpsimd.dma_start(wkq_t[:, 1:8, :], wkq_d[:, 1:8, :])
                nc.sync.dma_start(wb_t, wb_d.ap())
                for k in range(8):
                    nc.sync.dma_start(xt_t[:, k, 0:D], xt_d[:, k, 0:D])
                    if 0 < k < 6:
                        nc.scalar.dma_start(xt_t[:, k, D:S],
                                            xt_d[:, k, D:S])
                # late b-halves + wv on the SWDGE ring, gated on xt chunk
                # 2's arrival so they don't steal HBM bandwidth from the
                # kq-critical early chunks
                gate_t = const.tile([128, 1], BF16)
                nc.gpsimd.tensor_tensor(gate_t, xt_t[:, 2, 0:1],
                                        xt_t[:, 2, 0:1], op=AluOpType.mult)
                nc.gpsimd.dma_start(xt_t[:, 6, D:S], xt_d[:, 6, D:S])
                nc.gpsimd.dma_start(xt_t[:, 7, D:S], xt_d[:, 7, D:S])
                for k in range(8):
                    nc.gpsimd.dma_start(wv_t[:, k, :], wv_d[:, k, :])
                emit_const_dmas()

                # kq + biases in a single k-pass (paced to the xt DMA):
                # 4 kq banks + 2 bias banks
                with ExitStack() as ph0a:
                    pkq = ph0a.enter_context(
                        tc.tile_pool(name="pkq", bufs=4, space="PSUM"))
                    pb = ph0a.enter_context(
                        tc.tile_pool(name="pb", bufs=2, space="PSUM"))
                    warm = ph0a.enter_context(
                        tc.tile_pool(name="warm", bufs=1, space="PSUM"))
                    # PE warm-up: the HAM clock gate needs ~3.4us of
                    # sustained matmul activity to lift the 1.2 GHz cold
                    # throttle.  Fill the DMA-startup window (~1-10us)
                    # with inert zero matmuls so the kq pass runs at
                    # 2.4 GHz the moment xt chunk 0 lands.
                    warm_src = xp.tile([128, 512], BF16, tag="wsrc")
                    nc.vector.memset(warm_src, 0.0)
                    warm_ps = warm.tile([128, 512], F32, tag="wps")
                    for _ in range(4):
                        nc.tensor.matmul(warm_ps, warm_src[:, 0:128],
                                         warm_src, start=True, stop=True)
                    pks = {st: pkq.tile([128, 512], F32, tag="kq",
                                        name=f"pk{st}") for st in range(4)}
                    # bias rows: base partition must be 0/32/64, so three
                    # rows share one bank and the fourth gets its own
                    pbt0 = pb.tile([65, 512], F32, tag="b", name="pb0")
                    pbt1 = pb.tile([1, 512], F32, tag="b", name="pb1")

                    def pb_row(st):
                        return (pbt0[32 * st:32 * st + 1, :] if st < 3
                                else pbt1)
                    for k in range(7):
                        for st in range(4):
                            sl = slice(st * SB, (st + 1) * SB)
                            nc.tensor.matmul(
                                pks[st], wkq_t[:, k, :], xt_t[:, k, sl],
                                start=(k == 0), stop=False)
                        for st in range(4):
                            sl = slice(st * SB, (st + 1) * SB)
                            nc.tensor.matmul(
                                pb_row(st), wb_t[:, k, :], xt_t[:, k, sl],
                                start=(k == 0), stop=False)
                    # last k-chunk per-st so st0's copies overlap st1-3's
                    # matmuls; copies balanced across scalar and vector
                    for st in range(4):
                        sl = slice(st * SB, (st + 1) * SB)
                        nc.tensor.matmul(pks[st], wkq_t[:, 7, :],
                                         xt_t[:, 7, sl],
                                         start=False, stop=not have_bkq)
                        nc.tensor.matmul(pb_row(st), wb_t[:, 7, :],
                                         xt_t[:, 7, sl],
                                         start=False, stop=not have_bb)
                        if have_bkq:
                            nc.tensor.matmul(pks[st], bkq_t, ones_t[:, sl],
                                             start=False, stop=True)
                        if have_bb:
                            nc.tensor.matmul(pb_row(st), bb_t,
                                             ones_t[:, sl],
                                             start=False, stop=True)
                        if st % 2 == 0:
                            nc.scalar.activation(tile_k[0:64, sl],
                                                 pks[st][0:64, :], ACTF.Copy)
                            nc.vector.tensor_copy(tile_q[0:64, sl],
                                                  pks[st][64:128, :])
                        else:
                            nc.vector.tensor_copy(tile_k[0:64, sl],
                                                  pks[st][0:64, :])
                            nc.scalar.activation(tile_q[0:64, sl],
                                                 pks[st][64:128, :],
                                                 ACTF.Copy)
                        nc.scalar.activation(tile_k[64:65, sl], pb_row(st),
                                             ACTF.Copy)

                # values: out[s-chunk,128 x d-512], stationary xT slices
                scores_env['ps_s'] = ph01.enter_context(
                    tc.tile_pool(name="ps_s", bufs=4, space="PSUM"))
                pv = ph0.enter_context(
                    tc.tile_pool(name="pv", bufs=4, space="PSUM"))

                def emit_values(sc):
                    pvt = [pv.tile([128, 512], F32, tag="v",
                                   name=f"pv{sc % 2}_{d2}")
                           for d2 in range(ND2)]
                    for k in range(8):
                        for dt2 in range(ND2):
                            nc.tensor.matmul(
                                pvt[dt2], xt_t[:, k, sc * 128:(sc + 1) * 128],
                                wv_t[:, k, dt2 * 512:(dt2 + 1) * 512],
                                start=(k == 0),
                                stop=(k == 7 and not have_bv))
                    for dt2 in range(ND2):
                        dsl = slice(dt2 * 512, (dt2 + 1) * 512)
                        if have_bv:
                            nc.tensor.matmul(pvt[dt2], ones_t[:, 0:128],
                                             bv_t[:, dsl],
                                             start=False, stop=True)
                        if (sc * ND2 + dt2) % 2 == 0:
                            nc.scalar.activation(
                                v_sb[:, sc, dsl], pvt[dt2], ACTF.Copy)
                        else:
                            nc.vector.tensor_copy(v_sb[:, sc, dsl], pvt[dt2])

                # scores(0) interleaved into the values tail in 4-matmul
                # groups so all 16 sigmoids drain before attention starts
                for sc in range(16):
                    if sc >= 8 and sc % 2 == 0:
                        emit_scores(0, range(2 * (sc - 8), 2 * (sc - 8) + 4))
                    emit_values(sc)

            # ================= phase 1: scores + attention + LN1 =========
            with ExitStack() as ph1:
                xin = ph1.enter_context(tc.tile_pool(name="xin", bufs=4))
                rpp = ph1.enter_context(tc.tile_pool(name="rpp", bufs=3))
                stat = ph1.enter_context(tc.tile_pool(name="stat", bufs=8))
                ps_a = ph1.enter_context(
                    tc.tile_pool(name="ps_a", bufs=4, space="PSUM"))

                x_tiles = {}

                def emit_xdma(c):
                    # on the SWDGE ring, behind wv -- keeps the HWDGE
                    # queues free for the phase-0 critical path
                    if c < NC:
                        xt_ = xin.tile([128, D], BF16, tag="x", name=f"x{c}")
                        nc.gpsimd.dma_start(xt_,
                                            x_d[c * 128:(c + 1) * 128, :])
                        x_tiles[c] = xt_

                emit_xdma(0)
                emit_xdma(1)
                for sb in range(NSB):
                    for l in range(4):
                        c = sb * 4 + l
                        if c == 8:
                            # w1 pushed mid-phase-1: after the early x_t
                            # chunks, well before phase 2 reads it
                            nc.gpsimd.dma_start(w1_t[:, 0:4, :],
                                                w1_d[:, 0:4, :])
                            nc.gpsimd.dma_start(w1_t[:, 4:8, :],
                                                w1_d[:, 4:8, :])
                        emit_xdma(c + 2)
                        if sb < NSB - 1:
                            # next superblock's scores, 4 at a time, so the
                            # sigmoids hide under this l-chunk's attention
                            emit_scores(sb + 1, range(4 * l, 4 * l + 4))
                        pa = [ps_a.tile([128, 512], F32, tag="a",
                                        name=f"pa{c % 2}_{d2}")
                              for d2 in range(ND2)]
                        for j in range(NC):
                            st_tile = strip_tiles[sb][j // 4]
                            for dt2 in range(ND2):
                                nc.tensor.matmul(
                                    pa[dt2],
                                    st_tile[:, j % 4, l * 128:(l + 1) * 128],
                                    v_sb[:, j, dt2 * 512:(dt2 + 1) * 512],
                                    start=(j == 0), stop=(j == NC - 1))
                        x_t = x_tiles[c]
                        rp = rpp.tile([128, D], F32, tag="rp",
                                      name=f"rp{c % 3}")
                        for dt2 in range(ND2):
                            dsl = slice(dt2 * 512, (dt2 + 1) * 512)
                            nc.vector.tensor_tensor(
                                rp[:, dsl], pa[dt2], x_t[:, dsl],
                                op=AluOpType.add)
                        st_t = stat.tile([128, 2, 6], F32, tag="bst",
                                         name=f"bst{c}")
                        for g in range(2):
                            nc.vector.bn_stats(st_t[:, g, :],
                                               rp[:, g * 512:(g + 1) * 512])
                        mv = stat.tile([128, 2], F32, tag="mv", name=f"mv{c}")
                        nc.vector.bn_aggr(mv, st_t)
                        rstd, negmb = emit_norm_consts(nc.gpsimd, stat, mv,
                                                       f"r1_{c}")
                        if have_gb:
                            t1 = rpp.tile([128, D], F32, tag="t1",
                                          name=f"t1{c % 3}")
                            nc.vector.scalar_tensor_tensor(
                                out=t1, in0=rp, scalar=mv[:, 0:1], in1=g1_b,
                                op0=AluOpType.subtract, op1=AluOpType.mult)
                            nc.vector.scalar_tensor_tensor(
                                out=res_sb[:, c, :], in0=t1, scalar=rstd,
                                in1=be1_b,
                                op0=AluOpType.mult, op1=AluOpType.add)
                        else:
                            nc.scalar.activation(
                                res_sb[:, c, :], rp, ACTF.Identity,
                                bias=negmb, scale=rstd)
                        emit_tr(c)

        # ================= phase 2: FF + LN2 =================
        with ExitStack() as ph2:
            rtp = ph2.enter_context(tc.tile_pool(name="rtp", bufs=4))
            f2 = ph2.enter_context(tc.tile_pool(name="f2", bufs=3))
            ostage = ph2.enter_context(tc.tile_pool(name="ostage", bufs=3))
            stat2 = ph2.enter_context(tc.tile_pool(name="stat2", bufs=8))
            ps_f = ph2.enter_context(
                tc.tile_pool(name="ps_f", bufs=4, space="PSUM"))

            def stage_ff(c):
                pf = [ps_f.tile([128, 512], F32, tag="f",
                                name=f"pf{c % 2}_{d2}") for d2 in range(ND2)]
                r2 = f2.tile([128, D], F32, tag="r2", name=f"r2{c % 3}")
                st_t = stat2.tile([128, 2, 6], F32, tag="bst2",
                                  name=f"bst2_{c}")
                last = (c == NC - 1)
                if last:
                    # half-outer so the first half's LN2 work overlaps the
                    # second half's matmuls -- shortens the kernel tail
                    for dt2 in range(ND2):
                        dsl = slice(dt2 * 512, (dt2 + 1) * 512)
                        for k in range(8):
                            nc.tensor.matmul(
                                pf[dt2], rT[c][:, k, :], w1_t[:, k, dsl],
                                start=(k == 0),
                                stop=(k == 7 and not have_b1))
                        if have_b1:
                            nc.tensor.matmul(pf[dt2], ones_t[:, 0:128],
                                             b1_t[:, dsl],
                                             start=False, stop=True)
                        nc.vector.scalar_tensor_tensor(
                            out=r2[:, dsl], in0=pf[dt2], scalar=zero_t,
                            in1=res_sb[:, c, dsl],
                            op0=AluOpType.max, op1=AluOpType.add)
                        nc.vector.bn_stats(st_t[:, dt2, :], r2[:, dsl])
                else:
                    for k in range(8):
                        for dt2 in range(ND2):
                            nc.tensor.matmul(
                                pf[dt2], rT[c][:, k, :],
                                w1_t[:, k, dt2 * 512:(dt2 + 1) * 512],
                                start=(k == 0),
                                stop=(k == 7 and not have_b1))
                    for dt2 in range(ND2):
                        dsl = slice(dt2 * 512, (dt2 + 1) * 512)
                        if have_b1:
                            nc.tensor.matmul(pf[dt2], ones_t[:, 0:128],
                                             b1_t[:, dsl],
                                             start=False, stop=True)
                        nc.vector.scalar_tensor_tensor(
                            out=r2[:, dsl], in0=pf[dt2], scalar=zero_t,
                            in1=res_sb[:, c, dsl],
                            op0=AluOpType.max, op1=AluOpType.add)
                    for g in range(2):
                        nc.vector.bn_stats(st_t[:, g, :],
                                           r2[:, g * 512:(g + 1) * 512])
                mv = stat2.tile([128, 2], F32, tag="mv2", name=f"mv2_{c}")
                nc.vector.bn_aggr(mv, st_t)
                o_t = ostage.tile([128, D], F32, tag="o", name=f"o{c % 3}")
                if c == NC - 1:
                    # last chunk: keep the whole chain on DVE (no
                    # cross-engine hops) so the kernel tail is short;
                    # runs in parallel with chunk NC-2's Pool chain
                    rstd = emit_rstd(nc.vector, stat2, mv[:, 1:2],
                                     f"r2_{c}", iters=1)
                    for h in range(4):
                        hsl = slice(h * 256, (h + 1) * 256)
                        nc.vector.tensor_scalar(
                            out=o_t[:, hsl], in0=r2[:, hsl],
                            scalar1=mv[:, 0:1], scalar2=rstd,
                            op0=AluOpType.subtract, op1=AluOpType.mult)
                        deng = nc.sync if h % 2 == 0 else nc.scalar
                        deng.dma_start(out_d[c * 128:(c + 1) * 128, hsl],
                                       o_t[:, hsl])
                else:
                    rstd, negmb = emit_norm_consts(nc.gpsimd, stat2, mv,
                                                   f"r2_{c}")
                    for h in range(2):
                        hsl = slice(h * 512, (h + 1) * 512)
                        nc.scalar.activation(
                            o_t[:, hsl], r2[:, hsl], ACTF.Identity,
                            bias=negmb, scale=rstd)
                        deng = nc.sync if h == 0 else nc.scalar
                        deng.dma_start(out_d[c * 128:(c + 1) * 128, hsl],
                                       o_t[:, hsl])

            for c in range(NC):
                stage_ff(c)

    nc.finalize()
    return nc


_PROGRAM_CACHE = {}


def _get_program(flags):
    if flags not in _PROGRAM_CACHE:
        _PROGRAM_CACHE[flags] = build_program(flags)
    return _PROGRAM_CACHE[flags]


def kernel(x, Wk, bk, Wq, bq, Wv, bv, Wb, bb, W1, b1, g1, be1):
    import math
    scale = 1.0 / math.sqrt(K)
    flags = (
        bool(np.any(bk) or np.any(bq)),
        bool(np.any(bb)),
        bool(np.any(bv)),
        bool(np.any(b1)),
        bool(np.any(np.asarray(g1) != 1.0) or np.any(be1)),
    )
    nc = _get_program(flags)
    bf = ml_dtypes.bfloat16

    def _prep(w):
        # [D, M] -> [128, 8, M] matching SBUF [partition, chunk, free]
        return np.ascontiguousarray(
            np.asarray(w, np.float32).reshape(8, 128, -1).transpose(1, 0, 2)
        ).astype(bf)

    wkq = _prep(np.concatenate(
        [np.asarray(Wk, np.float32) * scale, np.asarray(Wq, np.float32)],
        axis=1))
    wb_ = _prep(Wb)
    wv_ = _prep(Wv)
    w1_ = _prep(W1)
    onesrow = np.ones((1, S), dtype=bf)
    iden = np.eye(128, dtype=np.float32).astype(bf)
    bkq = np.concatenate(
        [np.asarray(bk, np.float32) * scale,
         np.asarray(bq, np.float32)])[None, :].astype(bf)
    bbr = np.asarray(bb, np.float32).reshape(1, 1).astype(bf)
    bvr = np.asarray(bv, np.float32)[None, :].astype(bf)
    b1r = np.asarray(b1, np.float32)[None, :].astype(bf)
    g1r = np.asarray(g1, np.float32)[None, :]
    be1r = np.asarray(be1, np.float32)[None, :]

    xf = np.asarray(x, np.float32)
    xt = np.ascontiguousarray(np.transpose(xf, (0, 2, 1)))      # [B,D,S]
    xt = np.ascontiguousarray(
        xt.reshape(B, 8, 128, S).transpose(0, 2, 1, 3)).astype(bf)
    xs = xf.astype(bf)                                          # [B,S,D]

    in_maps = []
    for b in range(B):
        in_maps.append(dict(
            xt=xt[b], x=xs[b], wkq=wkq, wb=wb_, wv=wv_, w1=w1_,
            onesrow=onesrow, iden=iden, g1=g1r, be1=be1r, bkq=bkq, bb=bbr,
            bv=bvr, b1=b1r))

    res = run_bass_kernel_spmd(nc, in_maps, list(range(NCORES)), trace=False)
    out = np.stack([res.results[b]["out"] for b in range(B)], axis=0)
    return out.astype(np.float32)
